# revision 1
# baseline (speedup 1.0000x reference)
"""AttentionPooling kernel for Trainium2 (8 NeuronCores, SPMD, no collectives).

reference math:
    scores = tanh(x @ W1 + b1) @ W2 + b2        # [N, 1]
    attn   = softmax(scores, axis=0)            # global over all N rows
    pooled = segment_sum(x * attn, batch, 1024) # [1024, 256]

Strategy:
  - batch is sorted, so shard ROWS at graph boundaries: core c gets all rows
    with batch in [128c, 128(c+1)).  Each core owns exactly 128 output graphs
    -> no cross-core reduction for pooled.
  - b2 cancels in softmax (constant shift) -> dropped.  b1 handled by an extra
    rank-1 matmul only if nonzero (it is zeros in the reference data).
  - softmax normalizer: each core returns unnormalized A_g = sum_i e_i x_i and
    per-graph e-sums; host divides by the global Z (exact).
  - per 128-row tile on device:
      xT   = transpose(x_tile) on PE            (PSUM->SBUF copy on DVE+ACT)
      hT   = W1^T xT   (f32r matmuls, N=256)
      thT  = tanh(hT)  on ACT (PSUM->SBUF fused)
      s    = thT^T W2  (N=1 matmuls -> PSUM)
      e    = exp(s)    on ACT
      M    = (iota == brel) * e   one fused DVE tensor_scalar
      acc[128g, 256] += M^T @ x_tile   (f32r, N=256, PSUM-resident accumulator)
      esum[128g, 1]  += M^T @ ones
"""

import numpy as np
from contextlib import ExitStack

import concourse.bass as bass
import concourse.bacc as bacc
import concourse.mybir as mybir
import concourse.tile as tile
from concourse.bass_utils import run_bass_kernel_spmd
from concourse.masks import make_identity

F32 = mybir.dt.float32
F32R = mybir.dt.float32r
I32 = mybir.dt.int32

NUM_GRAPHS = 1024
NC = 8
GPC = NUM_GRAPHS // NC  # graphs per core = 128
P = 128
D = 256
ST = 8  # tiles per DMA supertile (1 MiB chunks)
SG = 2  # tiles per score group


def build_program(R: int, T: int, with_b1: bool) -> bass.Bass:
    assert T % ST == 0 and R == T * P
    nsup = T // ST

    nc = bacc.Bacc("TRN2", target_bir_lowering=False, debug=False)
    xs = nc.declare_dram_parameter("xs", [R, D], F32R, isOutput=False)
    brel = nc.declare_dram_parameter("brel", [P, T], F32, isOutput=False)
    w1 = nc.declare_dram_parameter("w1", [D, D], F32, isOutput=False)
    w2 = nc.declare_dram_parameter("w2", [P, 2], F32, isOutput=False)
    if with_b1:
        b1d = nc.declare_dram_parameter("b1d", [1, D], F32, isOutput=False)
    pooled = nc.declare_dram_parameter("pooled", [P, D], F32, isOutput=True)
    evec_out = nc.declare_dram_parameter("evec_out", [P, T], F32, isOutput=True)

    with ExitStack() as ctx:
        tc = ctx.enter_context(tile.TileContext(nc))
        const = ctx.enter_context(tc.tile_pool(name="const", bufs=1))
        xpool = ctx.enter_context(tc.tile_pool(name="x", bufs=3))
        xtpp = ctx.enter_context(tc.tile_pool(name="xtp", bufs=3, space="PSUM"))
        xtsp = ctx.enter_context(tc.tile_pool(name="xts", bufs=2))
        htpp = ctx.enter_context(tc.tile_pool(name="htp", bufs=2, space="PSUM"))
        thp = ctx.enter_context(tc.tile_pool(name="th", bufs=2))
        spp = ctx.enter_context(tc.tile_pool(name="sp", bufs=2, space="PSUM"))
        epl = ctx.enter_context(tc.tile_pool(name="e", bufs=2))
        mpl = ctx.enter_context(tc.tile_pool(name="m", bufs=3))
        accp = ctx.enter_context(tc.tile_pool(name="acc", bufs=1, space="PSUM"))
        outp = ctx.enter_context(tc.tile_pool(name="out", bufs=1))

        # ---- constants ----
        identf = const.tile([P, P], F32, tag="identf")
        make_identity(nc, identf[:])
        ident = const.tile([P, P], F32R)
        nc.vector.tensor_copy(ident[:], identf[:])
        iota_i = const.tile([P, P], I32)
        nc.gpsimd.iota(iota_i[:], pattern=[[1, P]], base=0, channel_multiplier=0)
        iota_f = const.tile([P, P], F32)
        nc.vector.tensor_copy(iota_f[:], iota_i[:])

        w1f = const.tile([P, 2, D], F32, tag="w1f")  # [d_lo, dc, j]
        nc.sync.dma_start(w1f[:], w1.rearrange("(dc p) j -> p dc j", p=P))
        w1sb = const.tile([P, 2, D], F32R)
        nc.vector.tensor_copy(w1sb[:], w1f[:])
        w2f = const.tile([P, 2], F32, tag="w2f")  # [j_lo, jc]
        nc.sync.dma_start(w2f[:], w2[:])
        # fp32r matmuls need moving free-dim >= 2: duplicate W2 column
        w2r = []
        for jc in range(2):
            t = const.tile([P, 2], F32R, tag=f"w2r{jc}")
            nc.vector.tensor_copy(t[:], w2f[:, jc : jc + 1].to_broadcast([P, 2]))
            w2r.append(t)
        brelsb = const.tile([P, T], F32)
        nc.sync.dma_start(brelsb[:], brel[:])
        if with_b1:
            b1f = const.tile([1, D], F32, tag="b1f")  # [1, j]
            nc.sync.dma_start(b1f[:], b1d[:])
            b1sb = const.tile([1, D], F32R)
            nc.vector.tensor_copy(b1sb[:], b1f[:])
            ones_rf = const.tile([1, SG * P], F32, tag="ones_rf")
            nc.gpsimd.memset(ones_rf[:], 1.0)
            ones_row = const.tile([1, SG * P], F32R)
            nc.vector.tensor_copy(ones_row[:], ones_rf[:])

        evec = const.tile([P, T], F32, tag="evec")  # exp(s) per row
        # persistent PSUM accumulator
        acc = accp.tile([P, D], F32)  # pooled[g, d]

        Tanh = mybir.ActivationFunctionType.Tanh
        Exp = mybir.ActivationFunctionType.Exp

        for sup in range(nsup):
            xsb = xpool.tile([P, ST, D], F32R)
            src = xs[sup * ST * P : (sup + 1) * ST * P, :]
            nc.sync.dma_start(xsb[:], src.rearrange("(t p) d -> p t d", p=P))

            for g in range(ST // SG):
                # onehot for each tile: independent of exp -> build early
                ohs = []
                for tt in range(SG):
                    gt = sup * ST + g * SG + tt
                    oh = mpl.tile([P, P], F32, tag="oh")
                    nc.vector.tensor_scalar(
                        oh[:],
                        iota_f[:],
                        brelsb[:, gt : gt + 1],
                        None,
                        op0=mybir.AluOpType.is_equal,
                    )
                    ohs.append(oh)
                # transpose SG tiles: xtp[d_lo, dc, tt, i]
                xtp = xtpp.tile([P, 2, SG, P], F32R)
                for tt in range(SG):
                    t = g * SG + tt
                    for dc in range(2):
                        nc.tensor.transpose(
                            xtp[:, dc, tt, :],
                            xsb[:, t, dc * P : (dc + 1) * P],
                            ident[:],
                        )
                xts = xtsp.tile([P, 2, SG, P], F32R)
                nc.vector.tensor_copy(xts[:, 0], xtp[:, 0])
                nc.vector.tensor_copy(xts[:, 1, 0], xtp[:, 1, 0])
                nc.scalar.copy(xts[:, 1, 1], xtp[:, 1, 1])

                # hT[j_lo, jc, i] = sum_d W1[d, j] xT[d, i]
                htp = htpp.tile([P, 2, SG * P], F32)
                for jc in range(2):
                    for dc in range(2):
                        nc.tensor.matmul(
                            htp[:, jc, :],
                            lhsT=w1sb[:, dc, jc * P : (jc + 1) * P],
                            rhs=xts[:, dc],
                            start=(dc == 0),
                            stop=(dc == 1 and not with_b1),
                        )
                    if with_b1:
                        nc.tensor.matmul(
                            htp[:, jc, :],
                            lhsT=b1sb[:, jc * P : (jc + 1) * P],
                            rhs=ones_row[:],
                            start=False,
                            stop=True,
                        )
                th = thp.tile([P, 2, SG * P], F32R)
                nc.scalar.activation(th[:], htp[:], Tanh)

                # s[i] per tile -> PSUM columns
                sp = spp.tile([P, SG, 2], F32)
                for tt in range(SG):
                    for jc in range(2):
                        nc.tensor.matmul(
                            sp[:, tt, :],
                            lhsT=th[:, jc, tt * P : (tt + 1) * P],
                            rhs=w2r[jc][:],
                            start=(jc == 0),
                            stop=(jc == 1),
                            skip_group_check=True,
                        )
                gt0 = sup * ST + g * SG
                nc.scalar.activation(evec[:, gt0 : gt0 + SG], sp[:, :, 0], Exp)

                for tt in range(SG):
                    t = g * SG + tt
                    gt = sup * ST + t
                    m = mpl.tile([P, P], F32R)
                    nc.vector.tensor_scalar(
                        m[:],
                        ohs[tt][:],
                        evec[:, gt : gt + 1],
                        None,
                        op0=mybir.AluOpType.mult,
                    )
                    nc.tensor.matmul(
                        acc[:],
                        lhsT=m[:],
                        rhs=xsb[:, t, :],
                        start=(gt == 0),
                        stop=(gt == T - 1),
                        skip_group_check=True,
                    )

        out_sb = outp.tile([P, D], F32)
        nc.vector.tensor_copy(out_sb[:], acc[:])
        nc.sync.dma_start(pooled[:], out_sb[:])
        nc.sync.dma_start(evec_out[:], evec[:])

    nc.compile()
    return nc


def _prep_inputs(x, batch, W1, b1, W2):
    """Shard rows at graph boundaries; pad to a common multiple of ST*P rows."""
    x = np.ascontiguousarray(np.asarray(x, dtype=np.float32))
    batch = np.asarray(batch)
    bounds = np.searchsorted(batch, np.arange(0, NUM_GRAPHS + 1, GPC))
    counts = np.diff(bounds)
    chunk = ST * P
    R = int(np.ceil(max(int(counts.max()), 1) / chunk) * chunk)
    T = R // P

    w1h = np.ascontiguousarray(np.asarray(W1, dtype=np.float32))  # [d, j]
    w2h = np.ascontiguousarray(
        np.asarray(W2, dtype=np.float32).reshape(2, P).transpose(1, 0)
    )  # -> [j_lo, jc]
    b1h = np.asarray(b1, dtype=np.float32).reshape(1, D)
    with_b1 = bool(np.any(b1h))

    in_maps = []
    for c in range(NC):
        lo, hi = int(bounds[c]), int(bounds[c + 1])
        n = hi - lo
        xs = np.zeros((R, D), dtype=np.float32)
        xs[:n] = x[lo:hi]
        br = np.full((R,), -1.0, dtype=np.float32)
        br[:n] = (np.asarray(batch[lo:hi], dtype=np.int64) - c * GPC).astype(
            np.float32
        )
        brel_pt = np.ascontiguousarray(br.reshape(T, P).transpose(1, 0))  # [P, T]
        m = {"xs": xs, "brel": brel_pt, "w1": w1h, "w2": w2h}
        if with_b1:
            m["b1d"] = b1h
        in_maps.append(m)
    return in_maps, R, T, with_b1, [int(c) for c in counts]


def run(x, batch, W1, b1, W2, b2, trace=False, trace_kwargs=None):
    in_maps, R, T, with_b1, counts = _prep_inputs(x, batch, W1, b1, W2)
    nc = build_program(R, T, with_b1)
    res = run_bass_kernel_spmd(
        nc,
        in_maps,
        core_ids=list(range(NC)),
        trace=trace,
        **(trace_kwargs or {}),
    )
    A = np.concatenate(
        [res.results[c]["pooled"] for c in range(NC)], axis=0
    ).astype(np.float64)
    Z = 0.0
    for c in range(NC):
        ev = res.results[c]["evec_out"].astype(np.float64)  # [P, T]
        n = counts[c]
        rows = ev.transpose(1, 0).reshape(-1)  # row r = t*128+p order
        Z += rows[:n].sum()
    out = (A / Z).astype(np.float32)
    return out, res


def kernel(x, batch, W1, b1, W2, b2):
    out, _ = run(x, batch, W1, b1, W2, b2)
    return out

